# revision 9
# baseline (speedup 1.0000x reference)
"""Grouped-correlation cost volume (CostVolume) Bass kernel for Trainium2.

Problem: x, y: (4, 512, 128, 256) f32; GROUP=4, MAXDISP=48, D=49.
out[b, g, k, h, w] = sum_cg x[b, 128g+cg, h, w] * y[b, 128g+cg, h, w-k]
(zero where w < k), out shape (4, 4, 49, 128, 256).

Strategy: shard the 16 (b, g) units over 8 cores (2 each; the channel sum is
within-group, so no cross-core reduce). Per (unit, h) row the correlation is
a banded Gram matrix between x columns and y columns with contraction over
cg = 128 = the TensorE partition dim. Each 128-wide w-block is split into
column groups of M x-columns (tile_position col-tiling) whose y-windows are
shifted by the group base:

  P[M*m+i', (M+48)*t+j'] = sum_cg x[cg, 128t+M*m+i'] * y[cg, 128t+M*m-48+j']

so the useful entries are j' = i' + 48 - k, an Mx(M+48) parallelogram per
group. The PSUM rows are copied to SBUF and stored to DRAM as-is; the band
extraction (a pure gather) happens on the host during the unshard step.

Geometry balances two near-equal rooflines (regime: ridge):
 - PE streaming work per row = sum of windows = 128 + 48*n_groups cycles
   per w-block (M=32: 640 cyc/row, M=64: 448). The PE is power/HAM
   throttled to ~50% duty for most of the run (~0.69 ns/cyc effective).
 - DMA = 32.6 MB of loads (irreducible) + stores = 2*128*(M+48) els/row
   (M=32: 41 KB/row, M=64: 57.3 KB/row) at a measured ~420 GB/s.
A 75%/25% row mix of M=32 (PE-heavy, DMA-light) and M=64 (PE-light,
DMA-heavy) finishes both engines together at ~110 us — better than either
pure geometry. Rows [0,192) use M=32 into staging outA; rows [192,256) use
M=64 into outB.

The whole pipeline runs in bf16 (inputs cast on host, matmul at 1 cyc/row,
output staged bf16) — the rel-err budget is 2e-2 and bf16 contributes
~3e-3. y is loaded contiguously (no padded rows): windows that reach before
y col 0 read garbage, which only lands in the w < k entries of the volume;
the host zeroes those after the band gather.

Scheduling: ~8 us of framework startup (barriers + ucode loads) precede
the first DMA; the first load chunk is small (8 rows) so the PE starts
~12 us in; chunks shrink at the end so the final drain is short. DMA work
is split evenly over the two HWDGE rings (x loads -> SP ring, y loads ->
Act ring, stores alternate) because one ring alone tops out ~315-360 GB/s
while both together reach ~420+. Stores go in 16-row blocks (<= 1.2 MB):
the SWDGE (gpsimd) queue only gets ~60 GB/s while HWDGE rings are busy,
and big blocks back up the copies, then PSUM, then the PE (which also
re-triggers the HAM cold throttle on every >3.4 us PE idle gap). PSUM
tiles hold 2 rows so one CAST/COPY instruction retires 2 rows; copies
alternate DVE/Act so neither saturates.

The module is built through bacc (not raw bass) so excess semaphore waits
get split onto EventSemaphore instructions.
"""

import os

import numpy as np
import ml_dtypes

import concourse.bass as bass
import concourse.mybir as mybir
import concourse.tile as tile
from concourse import bacc

MAXDISP = 48
D = MAXDISP + 1          # 49 disparities
CG = 128                 # channels per group = contraction dim
GROUP = 4
B = 4
H = 128
W = 256
NB = W // 128            # 2 w-blocks of 128
N_CORES = 8
N_UNITS = 2              # (b,g) units per core

MW_A, NWIN_A = 32, 80    # geometry A: M=32
MW_B, NWIN_B = 64, 112   # geometry B: M=64
RECT_A = NB * NWIN_A     # 160
RECT_B = NB * NWIN_B     # 224
SPLIT = 176              # global rows [0,SPLIT) -> A, [SPLIT,256) -> B

# load-chunk row counts per unit (sum 128 each): small first chunk so the
# PE starts early, ramp out at the end so the final drain is short
CHUNKS = {
    0: [8, 24, 32, 32, 32],
    1: [32, 32, 32, 16, 8, 4, 2, 2],
}
# store-block row counts: 16-row blocks in the bulk, per-chunk at the tail
STORES_A = [16] * 11                          # rows [0,176)
STORES_B = [16, 16, 16, 16, 8, 4, 2, 2]       # rows [176,256) -> outB [0,80)
MAX_CHUNK = 32

_last_results = None     # BassKernelResults of the most recent run (for test.py)


def build_nc(init_y_prefix=False):
    """Build the per-core Bass module (bf16).

    init_y_prefix memsets the 48-col garbage prefix of each y tile (needed
    only under CoreSim, which faults on uninitialized reads; hardware
    tolerates the garbage and the host zeroes the affected outputs).
    """
    bf16 = mybir.dt.bfloat16
    f32 = mybir.dt.float32
    y_len = MAXDISP + MAX_CHUNK * W   # 48-col garbage prefix + contiguous rows

    nc = bacc.Bacc()
    x = nc.dram_tensor("x", [N_UNITS, CG, H, W], bf16, kind="ExternalInput")
    y = nc.dram_tensor("y", [N_UNITS, CG, H, W], bf16, kind="ExternalInput")
    # flat row-major staging per geometry: out*[p, row, :]
    outA = nc.dram_tensor("outA", [128, SPLIT, RECT_A], bf16, kind="ExternalOutput")
    outB = nc.dram_tensor(
        "outB", [128, N_UNITS * H - SPLIT, RECT_B], bf16, kind="ExternalOutput"
    )

    assert all(sum(c) == H for c in CHUNKS.values())
    assert sum(STORES_A) == SPLIT and sum(STORES_B) == N_UNITS * H - SPLIT

    with tile.TileContext(nc) as tc:
        with (
            tc.tile_pool(name="io", bufs=3) as io_pool,
            tc.tile_pool(name="ybufs", bufs=3) as y_pool,
            tc.tile_pool(name="st", bufs=6) as st_pool,
            tc.tile_pool(name="psum_mm", bufs=8, space="PSUM") as psum_mm,
        ):
            # store-block state: (tile, tensor, row0, block_len, filled, rect)
            storesA, storesB = iter(STORES_A), iter(STORES_B)
            blk = None
            n_blocks = 0

            def next_block(gr):
                if gr < SPLIT:
                    n = next(storesA)
                    tl = st_pool.tile([128, 16, RECT_A], bf16, name="sA", tag="S")
                    return (tl, outA, gr, n, 0, RECT_A)
                n = next(storesB)
                tl = st_pool.tile([128, 16, RECT_B], bf16, name="sB", tag="S")
                return (tl, outB, gr - SPLIT, n, 0, RECT_B)

            for u in range(N_UNITS):
                h0 = 0
                for sz in CHUNKS[u]:
                    x_tile = io_pool.tile(
                        [128, MAX_CHUNK, W], bf16, name="x_tile", tag="x"
                    )
                    nc.sync.dma_start(
                        out=x_tile[:, :sz, :], in_=x[u, :, h0 : h0 + sz, :]
                    )

                    y_tile = y_pool.tile([128, y_len], bf16, name="y_tile", tag="y")
                    if init_y_prefix:
                        nc.vector.memset(y_tile[:, 0:MAXDISP], 0.0)
                    # y rows land contiguously at [48, 48 + sz*W)
                    y_dst = bass.AP(
                        tensor=y_tile.tensor,
                        offset=y_tile.offset + MAXDISP,
                        ap=[[y_len, 128], [W, sz], [1, W]],
                    )
                    # y on the Act HWDGE ring: each ring tops out ~315-360
                    # GB/s alone but ~420+ combined, so loads must be split
                    # across both rings (x -> SP, y -> Act)
                    nc.scalar.dma_start(out=y_dst, in_=y[u, :, h0 : h0 + sz, :])

                    for h in range(0, sz, 2):
                        gr = u * H + h0 + h
                        if gr < SPLIT:
                            mw, nwin, rect = MW_A, NWIN_A, RECT_A
                        else:
                            mw, nwin, rect = MW_B, NWIN_B, RECT_B
                        nm = 128 // mw
                        if blk is None:
                            blk = next_block(gr)
                        blk_tile, blk_t, blk_r0, blk_len, blk_fill, blk_rect = blk
                        # 2 rows per PSUM tile -> one cast retires 2 rows
                        p_mm = psum_mm.tile(
                            [128, 2, RECT_B], f32, name="p_mm", tag="P"
                        )
                        for hh in range(2):
                            for t in range(NB):
                                for m in range(nm):
                                    base = 128 * t + mw * m
                                    lhsT = x_tile[:, h + hh, base : base + mw]
                                    # window = y cols [base-48, base+mw) at
                                    # tile cols [(h+hh)*W + base, +nwin)
                                    c0 = (h + hh) * W + base
                                    nc.tensor.matmul(
                                        p_mm[mw * m : mw * (m + 1), hh,
                                             nwin * t : nwin * (t + 1)],
                                        lhsT,
                                        y_tile[:, c0 : c0 + nwin],
                                        start=True,
                                        stop=True,
                                        tile_position=(0, mw * m),
                                    )
                        dst = blk_tile[:, blk_fill : blk_fill + 2, :blk_rect]
                        # alternate copy engines so neither DVE nor Act
                        # saturates
                        if (h // 2) % 2 == 0:
                            nc.vector.tensor_copy(dst, p_mm[:, :, :rect])
                        else:
                            nc.scalar.copy(dst, p_mm[:, :, :rect])
                        blk_fill += 2
                        if blk_fill == blk_len:
                            # stores alternate between the two HWDGE rings to
                            # keep the per-ring byte totals balanced
                            st_eng = nc.sync if n_blocks % 2 == 0 else nc.scalar
                            n_blocks += 1
                            st_eng.dma_start(
                                out=blk_t[:, blk_r0 : blk_r0 + blk_len, :],
                                in_=blk_tile[:, :blk_len, :blk_rect],
                            )
                            blk = None
                        else:
                            blk = (
                                blk_tile, blk_t, blk_r0, blk_len, blk_fill,
                                blk_rect,
                            )
                    h0 += sz

    nc.finalize()
    return nc


def _shard_inputs(x, y):
    """x, y: (4, 512, 128, 256) bf16 -> per-core dicts of (2, 128, 128, 256)."""
    xu = x.reshape(B * GROUP, CG, H, W)
    yu = y.reshape(B * GROUP, CG, H, W)
    in_maps = []
    for c in range(N_CORES):
        in_maps.append(
            {
                "x": np.ascontiguousarray(xu[2 * c : 2 * c + 2]),
                "y": np.ascontiguousarray(yu[2 * c : 2 * c + 2]),
            }
        )
    return in_maps


def _extract(rect, mw, nwin):
    """rect: (n, 128, nR, NB*nwin) staging -> (n, nR, D, W) cd-indexed volume.

    rect[c, mw*m+i, row, nwin*t+j] = corr(x col 128t+mw*m+i, y col
    128t+mw*m-48+j) for that row; useful where j = i + cd, cd in [0, 48].
    """
    n, _, nR, _ = rect.shape
    nm = 128 // mw
    r = rect.reshape(n, nm, mw, nR, NB, nwin).transpose(0, 3, 1, 2, 4, 5)
    # dims [c, row, m, i, t, j]; select j = i + cd
    idx = np.arange(mw)[:, None] + np.arange(D)[None, :]
    g = np.take_along_axis(
        r, idx[None, None, None, :, None, :], axis=-1
    )  # [c, row, m, i, t, cd]
    g = g.transpose(0, 1, 5, 4, 2, 3)  # [c, row, cd, t, m, i]
    return g.reshape(n, nR, D, W)      # w = 128t + mw*m + i


def kernel(x, y):
    global _last_results
    from concourse.bass_utils import run_bass_kernel_spmd

    x = np.asarray(x, dtype=np.float32).astype(ml_dtypes.bfloat16)
    y = np.asarray(y, dtype=np.float32).astype(ml_dtypes.bfloat16)

    nc = build_nc()
    in_maps = _shard_inputs(x, y)
    trace = bool(int(os.environ.get("COSTVOL_TRACE", "0")))
    results = run_bass_kernel_spmd(
        nc,
        in_maps,
        core_ids=list(range(N_CORES)),
        trace=trace,
    )
    _last_results = results

    rA = np.stack([r["outA"] for r in results.results], axis=0)
    rB = np.stack([r["outB"] for r in results.results], axis=0)
    vA = _extract(rA, MW_A, NWIN_A)          # (n, SPLIT, D, W)
    vB = _extract(rB, MW_B, NWIN_B)          # (n, 256-SPLIT, D, W)
    v = np.concatenate([vA, vB], axis=1)     # rows = u*128 + h
    n = v.shape[0]
    v = v.reshape(n, N_UNITS, H, D, W).transpose(0, 1, 3, 2, 4)
    v = v.reshape(n * N_UNITS, D, H, W)[:, ::-1]   # cd -> k = 48 - cd
    v = np.ascontiguousarray(v).astype(np.float32)
    # zero the out-of-range band (w < k): the kernel leaves garbage there
    for k in range(1, D):
        v[:, k, :, :k] = 0.0
    return v.reshape(B, GROUP, D, H, W)


# revision 15
# speedup vs baseline: 1.0032x; 1.0032x over previous
"""Grouped-correlation cost volume (CostVolume) Bass kernel for Trainium2.

Problem: x, y: (4, 512, 128, 256) f32; GROUP=4, MAXDISP=48, D=49.
out[b, g, k, h, w] = sum_cg x[b, 128g+cg, h, w] * y[b, 128g+cg, h, w-k]
(zero where w < k), out shape (4, 4, 49, 128, 256).

Strategy: shard the 16 (b, g) units over 8 cores (2 each; the channel sum is
within-group, so no cross-core reduce). Per (unit, h) row the correlation is
a banded Gram matrix between x columns and y columns with contraction over
cg = 128 = the TensorE partition dim. Each 128-wide w-block is split into
column groups of M x-columns (tile_position col-tiling) whose y-windows are
shifted by the group base:

  P[M*m+i', (M+48)*t+j'] = sum_cg x[cg, 128t+M*m+i'] * y[cg, 128t+M*m-48+j']

so the useful entries are j' = i' + 48 - k, an Mx(M+48) parallelogram per
group. The PSUM rows are copied to SBUF and stored to DRAM as-is; the band
extraction (a pure gather) happens on the host during the unshard step.

Geometry balances two near-equal rooflines (regime: ridge):
 - PE streaming work per row = sum of windows = 128 + 48*n_groups cycles
   per w-block (M=32: 640 cyc/row, M=64: 448). The PE is power/HAM
   throttled to ~50% duty for most of the run (~0.69 ns/cyc effective).
 - DMA = 32.6 MB of loads (irreducible) + stores = 2*128*(M+48) els/row
   (M=32: 41 KB/row, M=64: 57.3 KB/row) at a measured ~420 GB/s.
The row mix of M=32 (PE-heavy, DMA-light) and M=64 (PE-light, DMA-heavy)
is chosen so the PE path stays at-or-below the DMA path even if the HAM
clock gate keeps the PE fully cold (1.2 GHz): the PE idles on loads
whenever DMA is the laggard, each >3.4 us idle re-throttles it, and a
cold PE slower than DMA convoys the whole pipeline. Rows [0,SPLIT) use
M=32 into staging outA; the rest use M=64 into outB.

The whole pipeline runs in bf16 (inputs cast on host, matmul at 1 cyc/row,
output staged bf16) — the rel-err budget is 2e-2 and bf16 contributes
~3e-3. y is loaded contiguously (no padded rows): windows that reach before
y col 0 read garbage, which only lands in the w < k entries of the volume;
the host zeroes those after the band gather.

Scheduling: ~8 us of framework startup (barriers + ucode loads) precede
the first DMA; the first load chunk is small (8 rows) so the PE starts
~12 us in; chunks shrink at the end so the final drain is short. DMA work
is split evenly over the two HWDGE rings (x loads -> SP ring, y loads ->
Act ring, stores alternate) because one ring alone tops out ~315-360 GB/s
while both together reach ~420+. Stores go in 16-row blocks (<= 1.2 MB):
the SWDGE (gpsimd) queue only gets ~60 GB/s while HWDGE rings are busy,
and big blocks back up the copies, then PSUM, then the PE (which also
re-triggers the HAM cold throttle on every >3.4 us PE idle gap). PSUM
tiles hold 2 rows so one CAST instruction retires 2 rows; all copies run
on DVE so the Act engine's strict-FIFO queue never delays its DMA issues.

The module is built through bacc (not raw bass) so excess semaphore waits
get split onto EventSemaphore instructions.
"""

import os

import numpy as np
import ml_dtypes

import concourse.bass as bass
import concourse.mybir as mybir
import concourse.tile as tile
from concourse import bacc

MAXDISP = 48
D = MAXDISP + 1          # 49 disparities
CG = 128                 # channels per group = contraction dim
GROUP = 4
B = 4
H = 128
W = 256
NB = W // 128            # 2 w-blocks of 128
N_CORES = 8
N_UNITS = 2              # (b,g) units per core

MW_A, NWIN_A = 32, 80    # geometry A: M=32
MW_B, NWIN_B = 64, 112   # geometry B: M=64
RECT_A = NB * NWIN_A     # 160
RECT_B = NB * NWIN_B     # 224
SPLIT = 80               # global rows [0,SPLIT) -> A, [SPLIT,256) -> B

# load-chunk row counts per unit (sum 128 each): small first chunk so the
# PE starts early, ramp out at the end so the final drain is short
CHUNKS = {
    0: [8, 24, 32, 32, 32],
    1: [32, 32, 32, 16, 8, 4, 2, 2],
}
# store-block row counts: 16-row blocks in the bulk, per-chunk at the tail
STORES_A = [16] * 5                           # rows [0,80)
STORES_B = [16] * 10 + [8, 4, 2, 2]           # rows [80,256) -> outB [0,176)
MAX_CHUNK = 32

_last_results = None     # BassKernelResults of the most recent run (for test.py)


def build_nc(init_y_prefix=False):
    """Build the per-core Bass module (bf16).

    init_y_prefix memsets the 48-col garbage prefix of each y tile (needed
    only under CoreSim, which faults on uninitialized reads; hardware
    tolerates the garbage and the host zeroes the affected outputs).
    """
    bf16 = mybir.dt.bfloat16
    f32 = mybir.dt.float32
    y_len = MAXDISP + MAX_CHUNK * W   # 48-col garbage prefix + contiguous rows

    nc = bacc.Bacc()
    x = nc.dram_tensor("x", [N_UNITS, CG, H, W], bf16, kind="ExternalInput")
    y = nc.dram_tensor("y", [N_UNITS, CG, H, W], bf16, kind="ExternalInput")
    # flat row-major staging per geometry: out*[p, row, :]
    outA = nc.dram_tensor("outA", [128, SPLIT, RECT_A], bf16, kind="ExternalOutput")
    outB = nc.dram_tensor(
        "outB", [128, N_UNITS * H - SPLIT, RECT_B], bf16, kind="ExternalOutput"
    )

    assert all(sum(c) == H for c in CHUNKS.values())
    assert sum(STORES_A) == SPLIT and sum(STORES_B) == N_UNITS * H - SPLIT

    with tile.TileContext(nc) as tc:
        with (
            tc.tile_pool(name="io", bufs=4) as io_pool,
            tc.tile_pool(name="ybufs", bufs=4) as y_pool,
            tc.tile_pool(name="st", bufs=6) as st_pool,
            tc.tile_pool(name="psum_mm", bufs=8, space="PSUM") as psum_mm,
        ):
            # store-block state: (tile, tensor, row0, block_len, filled, rect)
            storesA, storesB = iter(STORES_A), iter(STORES_B)
            blk = None
            n_blocks = 0

            def next_block(gr):
                if gr < SPLIT:
                    n = next(storesA)
                    tl = st_pool.tile([128, 16, RECT_A], bf16, name="sA", tag="S")
                    return (tl, outA, gr, n, 0, RECT_A)
                n = next(storesB)
                tl = st_pool.tile([128, 16, RECT_B], bf16, name="sB", tag="S")
                return (tl, outB, gr - SPLIT, n, 0, RECT_B)

            for u in range(N_UNITS):
                h0 = 0
                for sz in CHUNKS[u]:
                    x_tile = io_pool.tile(
                        [128, MAX_CHUNK, W], bf16, name="x_tile", tag="x"
                    )
                    nc.sync.dma_start(
                        out=x_tile[:, :sz, :], in_=x[u, :, h0 : h0 + sz, :]
                    )

                    y_tile = y_pool.tile([128, y_len], bf16, name="y_tile", tag="y")
                    if init_y_prefix:
                        nc.vector.memset(y_tile[:, 0:MAXDISP], 0.0)
                    # y rows land contiguously at [48, 48 + sz*W)
                    y_dst = bass.AP(
                        tensor=y_tile.tensor,
                        offset=y_tile.offset + MAXDISP,
                        ap=[[y_len, 128], [W, sz], [1, W]],
                    )
                    # y on the Act HWDGE ring: each ring tops out ~315-360
                    # GB/s alone but ~420+ combined, so loads must be split
                    # across both rings (x -> SP, y -> Act)
                    nc.scalar.dma_start(out=y_dst, in_=y[u, :, h0 : h0 + sz, :])

                    for h in range(0, sz, 2):
                        gr = u * H + h0 + h
                        if gr < SPLIT:
                            mw, nwin, rect = MW_A, NWIN_A, RECT_A
                        else:
                            mw, nwin, rect = MW_B, NWIN_B, RECT_B
                        nm = 128 // mw
                        if blk is None:
                            blk = next_block(gr)
                        blk_tile, blk_t, blk_r0, blk_len, blk_fill, blk_rect = blk
                        # 2 rows per PSUM tile -> one cast retires 2 rows
                        p_mm = psum_mm.tile(
                            [128, 2, RECT_B], f32, name="p_mm", tag="P"
                        )
                        for hh in range(2):
                            for t in range(NB):
                                for m in range(nm):
                                    base = 128 * t + mw * m
                                    lhsT = x_tile[:, h + hh, base : base + mw]
                                    # window = y cols [base-48, base+mw) at
                                    # tile cols [(h+hh)*W + base, +nwin)
                                    c0 = (h + hh) * W + base
                                    nc.tensor.matmul(
                                        p_mm[mw * m : mw * (m + 1), hh,
                                             nwin * t : nwin * (t + 1)],
                                        lhsT,
                                        y_tile[:, c0 : c0 + nwin],
                                        start=True,
                                        stop=True,
                                        tile_position=(0, mw * m),
                                    )
                        dst = blk_tile[:, blk_fill : blk_fill + 2, :blk_rect]
                        # all copies on DVE: the Act engine must only issue
                        # DMAs, or its HWDGE ring starves behind 600ns copies
                        # (head-of-line blocking in the strict-FIFO queue)
                        nc.vector.tensor_copy(dst, p_mm[:, :, :rect])
                        blk_fill += 2
                        if blk_fill == blk_len:
                            # stores alternate between the two HWDGE rings to
                            # keep the per-ring byte totals balanced
                            st_eng = nc.sync if n_blocks % 2 == 0 else nc.scalar
                            n_blocks += 1
                            st_eng.dma_start(
                                out=blk_t[:, blk_r0 : blk_r0 + blk_len, :],
                                in_=blk_tile[:, :blk_len, :blk_rect],
                            )
                            blk = None
                        else:
                            blk = (
                                blk_tile, blk_t, blk_r0, blk_len, blk_fill,
                                blk_rect,
                            )
                    h0 += sz

    nc.finalize()
    return nc


def _shard_inputs(x, y):
    """x, y: (4, 512, 128, 256) bf16 -> per-core dicts of (2, 128, 128, 256)."""
    xu = x.reshape(B * GROUP, CG, H, W)
    yu = y.reshape(B * GROUP, CG, H, W)
    in_maps = []
    for c in range(N_CORES):
        in_maps.append(
            {
                "x": np.ascontiguousarray(xu[2 * c : 2 * c + 2]),
                "y": np.ascontiguousarray(yu[2 * c : 2 * c + 2]),
            }
        )
    return in_maps


def _extract(rect, mw, nwin):
    """rect: (n, 128, nR, NB*nwin) staging -> (n, nR, D, W) cd-indexed volume.

    rect[c, mw*m+i, row, nwin*t+j] = corr(x col 128t+mw*m+i, y col
    128t+mw*m-48+j) for that row; useful where j = i + cd, cd in [0, 48].
    """
    n, _, nR, _ = rect.shape
    nm = 128 // mw
    r = rect.reshape(n, nm, mw, nR, NB, nwin).transpose(0, 3, 1, 2, 4, 5)
    # dims [c, row, m, i, t, j]; select j = i + cd
    idx = np.arange(mw)[:, None] + np.arange(D)[None, :]
    g = np.take_along_axis(
        r, idx[None, None, None, :, None, :], axis=-1
    )  # [c, row, m, i, t, cd]
    g = g.transpose(0, 1, 5, 4, 2, 3)  # [c, row, cd, t, m, i]
    return g.reshape(n, nR, D, W)      # w = 128t + mw*m + i


def kernel(x, y):
    global _last_results
    from concourse.bass_utils import run_bass_kernel_spmd

    x = np.asarray(x, dtype=np.float32).astype(ml_dtypes.bfloat16)
    y = np.asarray(y, dtype=np.float32).astype(ml_dtypes.bfloat16)

    nc = build_nc()
    in_maps = _shard_inputs(x, y)
    trace = bool(int(os.environ.get("COSTVOL_TRACE", "0")))
    results = run_bass_kernel_spmd(
        nc,
        in_maps,
        core_ids=list(range(N_CORES)),
        trace=trace,
    )
    _last_results = results

    rA = np.stack([r["outA"] for r in results.results], axis=0)
    rB = np.stack([r["outB"] for r in results.results], axis=0)
    vA = _extract(rA, MW_A, NWIN_A)          # (n, SPLIT, D, W)
    vB = _extract(rB, MW_B, NWIN_B)          # (n, 256-SPLIT, D, W)
    v = np.concatenate([vA, vB], axis=1)     # rows = u*128 + h
    n = v.shape[0]
    v = v.reshape(n, N_UNITS, H, D, W).transpose(0, 1, 3, 2, 4)
    v = v.reshape(n * N_UNITS, D, H, W)[:, ::-1]   # cd -> k = 48 - cd
    v = np.ascontiguousarray(v).astype(np.float32)
    # zero the out-of-range band (w < k): the kernel leaves garbage there
    for k in range(1, D):
        v[:, k, :, :k] = 0.0
    return v.reshape(B, GROUP, D, H, W)


# revision 20
# speedup vs baseline: 1.0352x; 1.0318x over previous
"""Grouped-correlation cost volume (CostVolume) Bass kernel for Trainium2.

Problem: x, y: (4, 512, 128, 256) f32; GROUP=4, MAXDISP=48, D=49.
out[b, g, k, h, w] = sum_cg x[b, 128g+cg, h, w] * y[b, 128g+cg, h, w-k]
(zero where w < k), out shape (4, 4, 49, 128, 256).

Strategy: shard the 16 (b, g) units over 8 cores (2 each; the channel sum is
within-group, so no cross-core reduce). Per (unit, h) row the correlation is
a banded Gram matrix between x columns and y columns with contraction over
cg = 128 = the TensorE partition dim. Each 128-wide w-block is split into
column groups of M x-columns (tile_position col-tiling) whose y-windows are
shifted by the group base:

  P[M*m+i', (M+48)*t+j'] = sum_cg x[cg, 128t+M*m+i'] * y[cg, 128t+M*m-48+j']

so the useful entries are j' = i' + 48 - k, an Mx(M+48) parallelogram per
group. The PSUM rows are copied to SBUF and stored to DRAM as-is; the band
extraction (a pure gather) happens on the host during the unshard step.

Geometry balances two near-equal rooflines (regime: ridge):
 - PE streaming work per row = sum of windows = 128 + 48*n_groups cycles
   per w-block (M=32: 640 cyc/row, M=64: 448). The PE is power/HAM
   throttled to ~50% duty for most of the run (~0.69 ns/cyc effective).
 - DMA = 32.6 MB of loads (irreducible) + stores = 2*128*(M+48) els/row
   (M=32: 41 KB/row, M=64: 57.3 KB/row) at a measured ~420 GB/s.
The row mix of M=32 (PE-heavy, DMA-light) and M=64 (PE-light, DMA-heavy)
is chosen so the PE path stays at-or-below the DMA path even if the HAM
clock gate keeps the PE fully cold (1.2 GHz): the PE idles on loads
whenever DMA is the laggard, each >3.4 us idle re-throttles it, and a
cold PE slower than DMA convoys the whole pipeline. Rows [0,SPLIT) use
M=32 into staging outA; the rest use M=64 into outB.

The whole pipeline runs in bf16 (inputs cast on host, matmul at 1 cyc/row,
output staged bf16) — the rel-err budget is 2e-2 and bf16 contributes
~3e-3. y is loaded contiguously (no padded rows): windows that reach before
y col 0 read garbage, which only lands in the w < k entries of the volume;
the host zeroes those after the band gather.

Scheduling: ~8 us of framework startup (barriers + ucode loads) precede
the first DMA; the first load chunk is small (8 rows) so the PE starts
~12 us in; chunks shrink at the end so the final drain is short. DMA work
is split evenly over the two HWDGE rings (x loads -> SP ring, y loads ->
Act ring, stores alternate) because one ring alone tops out ~315-360 GB/s
while both together reach ~420+. Stores go in 16-row blocks (<= 1.2 MB):
the SWDGE (gpsimd) queue only gets ~60 GB/s while HWDGE rings are busy,
and big blocks back up the copies, then PSUM, then the PE (which also
re-triggers the HAM cold throttle on every >3.4 us PE idle gap). PSUM
tiles hold 2 rows so one CAST instruction retires 2 rows; all copies run
on DVE so the Act engine's strict-FIFO queue never delays its DMA issues.

The module is built through bacc (not raw bass) so excess semaphore waits
get split onto EventSemaphore instructions.
"""

import os

import numpy as np
import ml_dtypes

import concourse.bass as bass
import concourse.mybir as mybir
import concourse.tile as tile
from concourse import bacc

MAXDISP = 48
D = MAXDISP + 1          # 49 disparities
CG = 128                 # channels per group = contraction dim
GROUP = 4
B = 4
H = 128
W = 256
NB = W // 128            # 2 w-blocks of 128
N_CORES = 8
N_UNITS = 2              # (b,g) units per core

MW_A, NWIN_A = 32, 80    # geometry A: M=32
MW_B, NWIN_B = 64, 112   # geometry B: M=64
RECT_A = NB * NWIN_A     # 160
RECT_B = NB * NWIN_B     # 224
SPLIT = 256              # global rows [0,SPLIT) -> A, [SPLIT,256) -> B

# load-chunk row counts per unit (sum 128 each): small first chunk so the
# PE starts early, ramp out at the end so the final drain is short
CHUNKS = {
    0: [8, 24, 32, 32, 32],
    1: [32, 32, 32, 16, 8, 4, 2, 2],
}
# store-block row counts: 16-row blocks in the bulk, per-chunk at the tail
STORES_A = [16] * 15 + [8, 4, 2, 2]           # rows [0,256)
STORES_B = []                                 # no B rows at SPLIT=256
MAX_CHUNK = 32

_last_results = None     # BassKernelResults of the most recent run (for test.py)


def build_nc(init_y_prefix=False):
    """Build the per-core Bass module (bf16).

    init_y_prefix memsets the 48-col garbage prefix of each y tile (needed
    only under CoreSim, which faults on uninitialized reads; hardware
    tolerates the garbage and the host zeroes the affected outputs).
    """
    bf16 = mybir.dt.bfloat16
    f32 = mybir.dt.float32
    y_len = MAXDISP + MAX_CHUNK * W   # 48-col garbage prefix + contiguous rows

    nc = bacc.Bacc()
    x = nc.dram_tensor("x", [N_UNITS, CG, H, W], bf16, kind="ExternalInput")
    y = nc.dram_tensor("y", [N_UNITS, CG, H, W], bf16, kind="ExternalInput")
    # flat row-major staging per geometry: out*[p, row, :]
    outA = nc.dram_tensor("outA", [128, SPLIT, RECT_A], bf16, kind="ExternalOutput")
    outB = None
    if SPLIT < N_UNITS * H:
        outB = nc.dram_tensor(
            "outB", [128, N_UNITS * H - SPLIT, RECT_B], bf16, kind="ExternalOutput"
        )

    assert all(sum(c) == H for c in CHUNKS.values())
    assert sum(STORES_A) == SPLIT and sum(STORES_B) == N_UNITS * H - SPLIT

    with tile.TileContext(nc) as tc:
        with (
            tc.tile_pool(name="io", bufs=4) as io_pool,
            tc.tile_pool(name="ybufs", bufs=4) as y_pool,
            tc.tile_pool(name="st", bufs=6) as st_pool,
            tc.tile_pool(name="psum_mm", bufs=8, space="PSUM") as psum_mm,
        ):
            # PE pre-warm: ~8.6 us of dummy matmuls on a memset scratch while
            # the first chunks load, so the HAM clock gate (cold 1.2 GHz ->
            # warm 2.4 GHz after ~3.4 us of sustained activity) is already
            # released when the real matmuls start at ~13 us
            warm = io_pool.tile([128, 128], bf16, name="warm", tag="w")
            nc.vector.memset(warm, 0.0)
            p_warm = psum_mm.tile([128, 2, RECT_B], f32, name="p_warm", tag="P")
            for _ in range(128):
                nc.tensor.matmul(
                    p_warm[0:32, 0, 0:80],
                    warm[:, 0:32],
                    warm[:, 0:80],
                    start=True,
                    stop=True,
                    tile_position=(0, 0),
                )

            # store-block state: (tile, tensor, row0, block_len, filled, rect)
            storesA, storesB = iter(STORES_A), iter(STORES_B)
            blk = None
            n_blocks = 0

            def next_block(gr):
                if gr < SPLIT:
                    n = next(storesA)
                    tl = st_pool.tile([128, 16, RECT_A], bf16, name="sA", tag="S")
                    return (tl, outA, gr, n, 0, RECT_A)
                n = next(storesB)
                tl = st_pool.tile([128, 16, RECT_B], bf16, name="sB", tag="S")
                return (tl, outB, gr - SPLIT, n, 0, RECT_B)

            for u in range(N_UNITS):
                h0 = 0
                for sz in CHUNKS[u]:
                    x_tile = io_pool.tile(
                        [128, MAX_CHUNK, W], bf16, name="x_tile", tag="x"
                    )
                    nc.sync.dma_start(
                        out=x_tile[:, :sz, :], in_=x[u, :, h0 : h0 + sz, :]
                    )

                    y_tile = y_pool.tile([128, y_len], bf16, name="y_tile", tag="y")
                    if init_y_prefix:
                        nc.vector.memset(y_tile[:, 0:MAXDISP], 0.0)
                    # y rows land contiguously at [48, 48 + sz*W)
                    y_dst = bass.AP(
                        tensor=y_tile.tensor,
                        offset=y_tile.offset + MAXDISP,
                        ap=[[y_len, 128], [W, sz], [1, W]],
                    )
                    # y on the Act HWDGE ring: each ring tops out ~315-360
                    # GB/s alone but ~420+ combined, so loads must be split
                    # across both rings (x -> SP, y -> Act)
                    nc.scalar.dma_start(out=y_dst, in_=y[u, :, h0 : h0 + sz, :])

                    for h in range(0, sz, 2):
                        gr = u * H + h0 + h
                        if gr < SPLIT:
                            mw, nwin, rect = MW_A, NWIN_A, RECT_A
                        else:
                            mw, nwin, rect = MW_B, NWIN_B, RECT_B
                        nm = 128 // mw
                        if blk is None:
                            blk = next_block(gr)
                        blk_tile, blk_t, blk_r0, blk_len, blk_fill, blk_rect = blk
                        # 2 rows per PSUM tile -> one cast retires 2 rows
                        p_mm = psum_mm.tile(
                            [128, 2, RECT_B], f32, name="p_mm", tag="P"
                        )
                        for hh in range(2):
                            for t in range(NB):
                                for m in range(nm):
                                    base = 128 * t + mw * m
                                    lhsT = x_tile[:, h + hh, base : base + mw]
                                    # window = y cols [base-48, base+mw) at
                                    # tile cols [(h+hh)*W + base, +nwin)
                                    c0 = (h + hh) * W + base
                                    nc.tensor.matmul(
                                        p_mm[mw * m : mw * (m + 1), hh,
                                             nwin * t : nwin * (t + 1)],
                                        lhsT,
                                        y_tile[:, c0 : c0 + nwin],
                                        start=True,
                                        stop=True,
                                        tile_position=(0, mw * m),
                                    )
                        dst = blk_tile[:, blk_fill : blk_fill + 2, :blk_rect]
                        # all copies on DVE: the Act engine must only issue
                        # DMAs, or its HWDGE ring starves behind 600ns copies
                        # (head-of-line blocking in the strict-FIFO queue)
                        nc.vector.tensor_copy(dst, p_mm[:, :, :rect])
                        blk_fill += 2
                        if blk_fill == blk_len:
                            # stores alternate between the two HWDGE rings to
                            # keep the per-ring byte totals balanced
                            st_eng = nc.sync if n_blocks % 2 == 0 else nc.scalar
                            n_blocks += 1
                            st_eng.dma_start(
                                out=blk_t[:, blk_r0 : blk_r0 + blk_len, :],
                                in_=blk_tile[:, :blk_len, :blk_rect],
                            )
                            blk = None
                        else:
                            blk = (
                                blk_tile, blk_t, blk_r0, blk_len, blk_fill,
                                blk_rect,
                            )
                    h0 += sz

    nc.finalize()
    return nc


def _shard_inputs(x, y):
    """x, y: (4, 512, 128, 256) bf16 -> per-core dicts of (2, 128, 128, 256)."""
    xu = x.reshape(B * GROUP, CG, H, W)
    yu = y.reshape(B * GROUP, CG, H, W)
    in_maps = []
    for c in range(N_CORES):
        in_maps.append(
            {
                "x": np.ascontiguousarray(xu[2 * c : 2 * c + 2]),
                "y": np.ascontiguousarray(yu[2 * c : 2 * c + 2]),
            }
        )
    return in_maps


def _extract(rect, mw, nwin):
    """rect: (n, 128, nR, NB*nwin) staging -> (n, nR, D, W) cd-indexed volume.

    rect[c, mw*m+i, row, nwin*t+j] = corr(x col 128t+mw*m+i, y col
    128t+mw*m-48+j) for that row; useful where j = i + cd, cd in [0, 48].
    """
    n, _, nR, _ = rect.shape
    nm = 128 // mw
    r = rect.reshape(n, nm, mw, nR, NB, nwin).transpose(0, 3, 1, 2, 4, 5)
    # dims [c, row, m, i, t, j]; select j = i + cd
    idx = np.arange(mw)[:, None] + np.arange(D)[None, :]
    g = np.take_along_axis(
        r, idx[None, None, None, :, None, :], axis=-1
    )  # [c, row, m, i, t, cd]
    g = g.transpose(0, 1, 5, 4, 2, 3)  # [c, row, cd, t, m, i]
    return g.reshape(n, nR, D, W)      # w = 128t + mw*m + i


def kernel(x, y):
    global _last_results
    from concourse.bass_utils import run_bass_kernel_spmd

    x = np.asarray(x, dtype=np.float32).astype(ml_dtypes.bfloat16)
    y = np.asarray(y, dtype=np.float32).astype(ml_dtypes.bfloat16)

    nc = build_nc()
    in_maps = _shard_inputs(x, y)
    trace = bool(int(os.environ.get("COSTVOL_TRACE", "0")))
    results = run_bass_kernel_spmd(
        nc,
        in_maps,
        core_ids=list(range(N_CORES)),
        trace=trace,
    )
    _last_results = results

    rA = np.stack([r["outA"] for r in results.results], axis=0)
    v = _extract(rA, MW_A, NWIN_A)           # (n, SPLIT, D, W)
    if SPLIT < N_UNITS * H:
        rB = np.stack([r["outB"] for r in results.results], axis=0)
        vB = _extract(rB, MW_B, NWIN_B)      # (n, 256-SPLIT, D, W)
        v = np.concatenate([v, vB], axis=1)  # rows = u*128 + h
    n = v.shape[0]
    v = v.reshape(n, N_UNITS, H, D, W).transpose(0, 1, 3, 2, 4)
    v = v.reshape(n * N_UNITS, D, H, W)[:, ::-1]   # cd -> k = 48 - cd
    v = np.ascontiguousarray(v).astype(np.float32)
    # zero the out-of-range band (w < k): the kernel leaves garbage there
    for k in range(1, D):
        v[:, k, :, :k] = 0.0
    return v.reshape(B, GROUP, D, H, W)
